# revision 13
# baseline (speedup 1.0000x reference)
"""Trainium2 Bass kernel for NoTPAttention (dense transformer block:
fused QKV projection -> multi-head attention -> output projection).

Sharding (8 NeuronCores): core c handles batch b = c // 4 and the 4 heads
g = 4*(c % 4) .. 4*(c % 4)+3 (head-parallel tensor parallelism).  Each core
computes its heads' partial out-projection [S, H] in bf16; the host sums the
4 partials per batch in fp32 and adds the (folded) biases.

Numerics: all matmuls run in bf16 with fp32 PSUM accumulation.  Softmax is
computed without max-subtraction (scores are bounded, |s| < ~3.5) with the
normalization deferred to the attention *output*:
    attnT[d, q] = (sum_k v[k, d] * exp(sT[k, q])) / (sum_k exp(sT[k, q]))
The denominator: the 16 key-tiles of exp(sT) are pairwise-tree-summed on the
vector engine (4 strided in-place bf16 adds), then a single ones-matmul
broadcasts the cross-partition sum - 16x less tensor-engine work than a
full ones-matmul accumulation chain.  The v-bias is dropped in-kernel (it
contributes exactly b_v per row after normalization; the host folds
w_out @ b_v into the output bias).

Schedule: the PE instruction stream is emitted in "mini-steps" that weave
the score matmuls of chunk i between the PV matmuls of chunk i-2 and the
out-projection groups, so the in-order PE queue never stalls on the scalar
engine's exp pacing.  The first two chunks' score matmuls are woven into
the tail of the QKV phase (after their k/q slices complete) so the exp
stream gets a 2-chunk head start before PV consumption begins.  Startup:
8 warm-up matmuls on a memset ones tile lift the PE HAM clock gate to
8/8 before the first DMA-gated projection matmul issues.
"""

import numpy as np
import ml_dtypes

B, S, H = 2, 2048, 2048
NH, HD = 16, 128
P = 128
HT = H // P            # 16 hidden-dim tiles
G = 4                  # heads per core
GH = G * HD            # 512: head-group width per core
SCALE = 1.0 / float(np.sqrt(HD))
N_CORES = 8
XC = 512               # phase-1 x streaming chunk (s elements)
QC = 512               # attention query chunk
KT = S // P            # 16 key tiles
NXC = S // XC          # 4
NQC = S // QC          # 4
NCH = G * NQC          # 16 attention chunks

_CACHE = {}


def _build():
    import concourse.mybir as mybir
    import concourse.tile as tile
    from concourse import bacc

    dt = mybir.dt
    Alu = mybir.AluOpType
    Act = mybir.ActivationFunctionType

    nc = bacc.Bacc("TRN2", target_bir_lowering=False, debug=False,
                   enable_asserts=False)

    xt_d = nc.dram_tensor("xt", [H, S], dt.bfloat16, kind="ExternalInput").ap()
    wqt_d = nc.dram_tensor("wqt", [H, GH], dt.bfloat16, kind="ExternalInput").ap()
    wkt_d = nc.dram_tensor("wkt", [H, GH], dt.bfloat16, kind="ExternalInput").ap()
    wvt_d = nc.dram_tensor("wvt", [H, GH], dt.bfloat16, kind="ExternalInput").ap()
    bqs_d = nc.dram_tensor("bqs", [P, G], dt.float32, kind="ExternalInput").ap()
    bk_d = nc.dram_tensor("bk", [P, G], dt.float32, kind="ExternalInput").ap()
    wot_d = nc.dram_tensor("wot", [GH, H], dt.bfloat16, kind="ExternalInput").ap()
    out_d = nc.dram_tensor("partial", [S, H], dt.bfloat16, kind="ExternalOutput").ap()

    xt_r = xt_d.rearrange("(ht p) s -> p ht s", p=P)      # [128, 16, 2048]
    wqt_r = wqt_d.rearrange("(ht p) o -> p ht o", p=P)    # [128, 16, 512]
    wkt_r = wkt_d.rearrange("(ht p) o -> p ht o", p=P)
    wvt_r = wvt_d.rearrange("(ht p) o -> p ht o", p=P)
    wot_r = wot_d.rearrange("(g p) o -> p g o", p=P)      # [128, 4, 2048]

    with tile.TileContext(nc) as tc:
        with (
            tc.tile_pool(name="consts", bufs=1) as consts,
            tc.tile_pool(name="wpool", bufs=1) as wpool,
            tc.tile_pool(name="xpool", bufs=2) as xpool,
            tc.tile_pool(name="big", bufs=1) as big,
            tc.tile_pool(name="epool", bufs=4) as epool,
            tc.tile_pool(name="small", bufs=2) as small,
            tc.tile_pool(name="psum", bufs=2, space="PSUM") as psum,
        ):
            # ---- HAM warm-up: get the PE clock gate to 8/8 before the
            # first real (DMA-gated) matmul arrives ----
            ones_sb = consts.tile([P, 512], dt.bfloat16)
            nc.vector.memset(ones_sb[:], 1.0)
            warm_ps = psum.tile([P, 2, QC], dt.float32, tag="st", name="ps")
            for _ in range(12):
                nc.tensor.matmul(warm_ps[:, 0, :], ones_sb[:, 0:P], ones_sb[:],
                                 start=True, stop=True)

            # ---- startup DMAs, critical-path first: the first q-matmul
            # (head 0) needs wq cols 0:128 and x chunk 0 ----
            wq_sb = epool.tile([P, HT, GH], dt.bfloat16, tag="e", name="wq_sb")
            nc.sync.dma_start(wq_sb[:, :, 0:HD], wqt_r[:, :, 0:HD])
            xt0_sb = xpool.tile([P, HT, XC], dt.bfloat16, tag="xt",
                                name="xt0_sb")
            for q4 in range(4):
                nc.sync.dma_start(xt0_sb[:, 4 * q4:4 * (q4 + 1), :],
                                  xt_r[:, 4 * q4:4 * (q4 + 1), 0:XC])
            bqs_sb = consts.tile([P, G], dt.float32)
            nc.sync.dma_start(bqs_sb[:], bqs_d)
            bk_sb = consts.tile([P, G], dt.float32)
            nc.sync.dma_start(bk_sb[:], bk_d)
            nc.sync.dma_start(wq_sb[:, :, HD:], wqt_r[:, :, HD:])
            wk_sb = epool.tile([P, HT, GH], dt.bfloat16, tag="e", name="wk_sb")
            nc.sync.dma_start(wk_sb[:, :, 0:HD], wkt_r[:, :, 0:HD])
            nc.sync.dma_start(wk_sb[:, :, HD:], wkt_r[:, :, HD:])
            wv_sb = epool.tile([P, HT, GH], dt.bfloat16, tag="e", name="wv_sb")
            nc.sync.dma_start(wv_sb[:], wvt_r)

            qt_sb = big.tile([P, G, S], dt.bfloat16)   # q^T, scale+bias applied
            kt_sb = big.tile([P, G, S], dt.bfloat16)   # k^T, bias applied
            v_sb = big.tile([P, KT, GH], dt.bfloat16)  # v natural [s, o]
            at_sb = big.tile([P, G, S], dt.bfloat16)   # attn output^T

            # ---------------- phase-1 building blocks ----------------
            def q_group(xt_sb, xc, h):
                sl = slice(xc * XC, (xc + 1) * XC)
                psq = psum.tile([P, 512], dt.float32, tag="mm", name="psq")
                for ht in range(HT):
                    nc.tensor.matmul(psq,
                                     wq_sb[:, ht, h * HD:(h + 1) * HD],
                                     xt_sb[:, ht, :],
                                     start=(ht == 0), stop=(ht == HT - 1))
                nc.vector.tensor_scalar(qt_sb[:, h, sl], psq,
                                        SCALE, bqs_sb[:, h:h + 1],
                                        Alu.mult, Alu.add)

            def k_group(xt_sb, xc, h):
                sl = slice(xc * XC, (xc + 1) * XC)
                psk = psum.tile([P, 512], dt.float32, tag="mm", name="psk")
                for ht in range(HT):
                    nc.tensor.matmul(psk,
                                     wk_sb[:, ht, h * HD:(h + 1) * HD],
                                     xt_sb[:, ht, :],
                                     start=(ht == 0), stop=(ht == HT - 1))
                nc.vector.tensor_scalar_add(kt_sb[:, h, sl], psk,
                                            bk_sb[:, h:h + 1])

            def v_group(xt_sb, xc, sv):
                sm = xc * (XC // P) + sv
                psv = psum.tile([P, 512], dt.float32, tag="mm", name="psv")
                for ht in range(HT):
                    nc.tensor.matmul(psv,
                                     xt_sb[:, ht, sv * P:(sv + 1) * P],
                                     wv_sb[:, ht, :],
                                     start=(ht == 0), stop=(ht == HT - 1))
                nc.vector.tensor_copy(out=v_sb[:, sm, :], in_=psv)

            # ---------------- attention building blocks ----------------
            e_tiles = [None] * NCH
            pv_tiles = [None] * NCH
            zrow_tiles = [None] * NCH
            proj_q = []
            drain_flip = [0]

            def st_pair(idx, ms):
                # two 128-key score matmuls + one batched exp
                h, qc = idx % G, idx // G
                km = 2 * ms
                ps = psum.tile([P, 2, QC], dt.float32, tag="st", name="ps")
                for j in range(2):
                    nc.tensor.matmul(ps[:, j, :],
                                     kt_sb[:, h, (km + j) * P:(km + j + 1) * P],
                                     qt_sb[:, h, qc * QC:(qc + 1) * QC],
                                     start=True, stop=True)
                nc.scalar.activation(e_tiles[idx][:, km:km + 2, :], ps, Act.Exp)

            def pv_pair(idx, ms):
                h = idx % G
                pv, e = pv_tiles[idx], e_tiles[idx]
                for j in range(2):
                    km = 2 * ms + j
                    nc.tensor.matmul(pv, v_sb[:, km, h * HD:(h + 1) * HD],
                                     e[:, km, :],
                                     start=(km == 0), stop=(km == KT - 1))
                # level-1 of the denominator tree: fold this pair's odd tile
                # into the even one right after PV is done reading both
                nc.vector.tensor_add(e[:, 2 * ms, :], e[:, 2 * ms, :],
                                     e[:, 2 * ms + 1, :])
                if idx == NCH - 1 and ms % 2 == 1:
                    # last chunk: run the upper tree levels incrementally so
                    # the final zrow is ready right after the last PV pair
                    q4 = 2 * (ms - 1)
                    nc.vector.tensor_add(e[:, q4, :], e[:, q4, :],
                                         e[:, q4 + 2, :])
                    if ms == 3:
                        nc.vector.tensor_add(e[:, 0, :], e[:, 0, :],
                                             e[:, 4, :])
                    if ms == 7:
                        nc.vector.tensor_add(e[:, 8, :], e[:, 8, :],
                                             e[:, 12, :])
                        zr = small.tile([P, QC], dt.bfloat16, tag="zr",
                                        name="zr")
                        nc.vector.tensor_add(zr[:], e[:, 0, :], e[:, 8, :])
                        zrow_tiles[idx] = zr

            def tree_fin(idx):
                # levels 2-4 of the in-place pairwise bf16 tree; the final
                # sum lands in a small zrow buffer so the e-tile's last
                # reader is this tree, not the (later) ones-matmul
                e = e_tiles[idx]
                nc.vector.tensor_add(e[:, 0:KT:4, :], e[:, 0:KT:4, :],
                                     e[:, 2:KT:4, :])
                nc.vector.tensor_add(e[:, 0:KT:8, :], e[:, 0:KT:8, :],
                                     e[:, 4:KT:8, :])
                zr = small.tile([P, QC], dt.bfloat16, tag="zr", name="zr")
                nc.vector.tensor_add(zr[:], e[:, 0, :], e[:, 8, :])
                zrow_tiles[idx] = zr

            def zmm_norm(idx):
                # single ones-matmul: cross-partition sum of the tile-summed
                # exps, broadcast to all 128 partitions; then normalize
                h, qc = idx % G, idx // G
                z = psum.tile([P, QC], dt.float32, tag="mm", name="z")
                nc.tensor.matmul(z, ones_sb[:, 0:P], zrow_tiles[idx],
                                 start=True, stop=True)
                zi = small.tile([P, QC], dt.float32, tag="zi", name="zi")
                nc.vector.reciprocal_approx_fast(out=zi[:], in_=z)
                nc.vector.tensor_mul(out=at_sb[:, h, qc * QC:(qc + 1) * QC],
                                     in0=pv_tiles[idx], in1=zi[:])
                if h == G - 1:
                    for sv in range(QC // P):
                        for oc in range(H // 512):
                            proj_q.append((qc * (QC // P) + sv, oc))

            def proj_group(alt=False):
                if not proj_q:
                    return
                sm, oc = proj_q.pop(0)
                drain_flip[0] ^= 1
                pp = psum.tile([P, 512], dt.float32, tag="mm", name="pp")
                for g in range(G):
                    nc.tensor.matmul(pp,
                                     at_sb[:, g, sm * P:(sm + 1) * P],
                                     wo_sb[:, g, oc * 512:(oc + 1) * 512],
                                     start=(g == 0), stop=(g == G - 1))
                ob = small.tile([P, 512], dt.bfloat16, tag="ob", bufs=3,
                                name="ob")
                if alt and drain_flip[0]:
                    nc.scalar.copy(ob[:], pp)
                else:
                    nc.vector.tensor_copy(out=ob[:], in_=pp)
                nc.sync.dma_start(
                    out_d[sm * P:(sm + 1) * P, oc * 512:(oc + 1) * 512],
                    ob[:])

            # ---------------- Phase 1: QKV projections ----------------
            xts = [None] * NXC
            for xc in range(NXC):
                if xc == 0:
                    xts[xc] = xt0_sb
                else:
                    xts[xc] = xpool.tile([P, HT, XC], dt.bfloat16, tag="xt",
                                         name="xt_sb")
                    nc.sync.dma_start(xts[xc][:],
                                      xt_r[:, :, xc * XC:(xc + 1) * XC])
                if xc < NXC - 1:
                    for h in range(G):
                        q_group(xts[xc], xc, h)
                    if xc == 0:
                        # filler: the wk DMA lands ~5us after q(xc0) ends;
                        # keep the PE busy so the HAM clock gate stays 8/8
                        for _ in range(24):
                            nc.tensor.matmul(warm_ps[:, 1, :],
                                             ones_sb[:, 0:P], ones_sb[:],
                                             start=True, stop=True)
                    for h in range(G):
                        k_group(xts[xc], xc, h)
                    for sv in range(XC // P):
                        v_group(xts[xc], xc, sv)

            # last x chunk: k first, then weave the first two chunks' score
            # matmuls into the q/v groups so exp gets a head start
            xt3 = xts[NXC - 1]
            e_tiles[0] = epool.tile([P, KT, QC], dt.bfloat16, tag="e",
                                    name="e_sb")
            k_group(xt3, 3, 0)
            for h in range(1, G):
                k_group(xt3, 3, h)
                st_pair(0, 2 * (h - 1))
                st_pair(0, 2 * (h - 1) + 1)
            q_group(xt3, 3, 0)
            st_pair(0, 6)
            st_pair(0, 7)
            for h in range(1, G):
                q_group(xt3, 3, h)
            # wq now dead -> its pool slot is free for e_tiles[1]
            e_tiles[1] = epool.tile([P, KT, QC], dt.bfloat16, tag="e",
                                    name="e_sb")
            for sv in range(XC // P):
                v_group(xt3, 3, sv)
                st_pair(1, 2 * sv)
                st_pair(1, 2 * sv + 1)

            # out-proj weights: needed only from the first proj (~mid-kernel)
            wo_sb = wpool.tile([P, G, H], dt.bfloat16)
            nc.sync.dma_start(wo_sb[:], wot_r)

            # -------- Phase 2+3: mini-step interleaved chunk pipeline --------
            for j in range(2, 18):
                cur = j if j <= NCH - 1 else None
                pvi = j - 2 if j - 2 <= NCH - 1 else None
                zni = j - 3 if 0 <= j - 3 <= NCH - 1 else None
                if cur is not None:
                    e_tiles[cur] = epool.tile([P, KT, QC], dt.bfloat16,
                                              tag="e", name="e_sb")
                if pvi is not None:
                    pv_tiles[pvi] = psum.tile([P, QC], dt.float32, tag="pv",
                                              name="pv")
                for ms in range(8):
                    if cur is not None:
                        st_pair(cur, ms)
                    if pvi is not None:
                        pv_pair(pvi, ms)
                    if ms == 2 and zni is not None:
                        zmm_norm(zni)
                    if (ms % 2 == 1) and (cur is not None or ms < 7):
                        proj_group()
                if pvi is not None and pvi != NCH - 1:
                    tree_fin(pvi)
            proj_group(alt=True)
            proj_group(alt=True)
            zmm_norm(NCH - 1)
            while proj_q:
                proj_group(alt=True)

    nc.compile()
    return nc


def _get_nc():
    if "nc" not in _CACHE:
        _CACHE["nc"] = _build()
    return _CACHE["nc"]


def _make_in_maps(x, w_qkv, b_qkv, w_out):
    bf = ml_dtypes.bfloat16
    f32 = np.float32
    in_maps = []
    for c in range(N_CORES):
        b = c // 4
        g = c % 4
        lo = GH * g
        hi = GH * (g + 1)
        xt = np.ascontiguousarray(x[b].T).astype(bf)
        wqt = np.ascontiguousarray(w_qkv[lo:hi, :].T).astype(bf)
        wkt = np.ascontiguousarray(w_qkv[H + lo:H + hi, :].T).astype(bf)
        wvt = np.ascontiguousarray(w_qkv[2 * H + lo:2 * H + hi, :].T).astype(bf)
        bqs = np.ascontiguousarray(
            (b_qkv[lo:hi] * SCALE).astype(f32).reshape(G, P).T)
        bk = np.ascontiguousarray(
            b_qkv[H + lo:H + hi].astype(f32).reshape(G, P).T)
        wot = np.ascontiguousarray(w_out[:, lo:hi].T).astype(bf)
        in_maps.append({"xt": xt, "wqt": wqt, "wkt": wkt, "wvt": wvt,
                        "bqs": bqs, "bk": bk, "wot": wot})
    return in_maps


def kernel(x, w_qkv, b_qkv, w_out, b_out):
    import os
    import sys

    x = np.asarray(x, dtype=np.float32)
    w_qkv = np.asarray(w_qkv, dtype=np.float32)
    b_qkv = np.asarray(b_qkv, dtype=np.float32)
    w_out = np.asarray(w_out, dtype=np.float32)
    b_out = np.asarray(b_out, dtype=np.float32)

    from concourse.bass_utils import run_bass_kernel_spmd

    # NTFF tracing under axon needs the antenv.axon_hooks shim (test.py
    # installs it); without it a stray BASS_TRACE=1 in the environment would
    # crash the run — disable tracing in that case.
    if "antenv.axon_hooks" not in sys.modules:
        os.environ["BASS_NEVER_TRACE"] = "1"

    nc = _get_nc()
    in_maps = _make_in_maps(x, w_qkv, b_qkv, w_out)
    res = run_bass_kernel_spmd(nc, in_maps, core_ids=list(range(N_CORES)))
    _CACHE["last_results"] = res
    partials = [r["partial"] for r in res.results]

    bv = b_qkv[2 * H:3 * H]
    bias = b_out + w_out @ bv          # folded v-bias contribution
    out = np.empty((B, S, H), np.float32)
    for b in range(B):
        acc = partials[4 * b].astype(np.float32)
        for g in range(1, 4):
            acc += partials[4 * b + g].astype(np.float32)
        out[b] = acc + bias
    return out


# revision 15
# speedup vs baseline: 1.0099x; 1.0099x over previous
"""Trainium2 Bass kernel for NoTPAttention (dense transformer block:
fused QKV projection -> multi-head attention -> output projection).

Sharding (8 NeuronCores): core c handles batch b = c // 4 and the 4 heads
g = 4*(c % 4) .. 4*(c % 4)+3 (head-parallel tensor parallelism).  Each core
computes its heads' partial out-projection [S, H] in bf16; the host sums the
4 partials per batch in fp32 and adds the (folded) biases.

Numerics: all matmuls run in bf16 with fp32 PSUM accumulation.  Softmax is
computed without max-subtraction (scores are bounded, |s| < ~3.5) with the
normalization deferred to the attention *output*:
    attnT[d, q] = (sum_k v[k, d] * exp(sT[k, q])) / (sum_k exp(sT[k, q]))
The denominator: the 16 key-tiles of exp(sT) are pairwise-tree-summed on the
vector engine (4 strided in-place bf16 adds), then a single ones-matmul
broadcasts the cross-partition sum - 16x less tensor-engine work than a
full ones-matmul accumulation chain.  The v-bias is dropped in-kernel (it
contributes exactly b_v per row after normalization; the host folds
w_out @ b_v into the output bias).

Schedule: the PE instruction stream is emitted in "mini-steps" that weave
the score matmuls of chunk i between the PV matmuls of chunk i-2 and the
out-projection groups, so the in-order PE queue never stalls on the scalar
engine's exp pacing.  The first two chunks' score matmuls are woven into
the tail of the QKV phase (after their k/q slices complete) so the exp
stream gets a 2-chunk head start before PV consumption begins.  Startup:
8 warm-up matmuls on a memset ones tile lift the PE HAM clock gate to
8/8 before the first DMA-gated projection matmul issues.
"""

import numpy as np
import ml_dtypes

B, S, H = 2, 2048, 2048
NH, HD = 16, 128
P = 128
HT = H // P            # 16 hidden-dim tiles
G = 4                  # heads per core
GH = G * HD            # 512: head-group width per core
SCALE = 1.0 / float(np.sqrt(HD))
N_CORES = 8
XC = 512               # phase-1 x streaming chunk (s elements)
QC = 512               # attention query chunk
KT = S // P            # 16 key tiles
NXC = S // XC          # 4
NQC = S // QC          # 4
NCH = G * NQC          # 16 attention chunks

_CACHE = {}


def _build():
    import concourse.mybir as mybir
    import concourse.tile as tile
    from concourse import bacc

    dt = mybir.dt
    Alu = mybir.AluOpType
    Act = mybir.ActivationFunctionType

    nc = bacc.Bacc("TRN2", target_bir_lowering=False, debug=False,
                   enable_asserts=False)

    xt_d = nc.dram_tensor("xt", [H, S], dt.bfloat16, kind="ExternalInput").ap()
    wqt_d = nc.dram_tensor("wqt", [H, GH], dt.bfloat16, kind="ExternalInput").ap()
    wkt_d = nc.dram_tensor("wkt", [H, GH], dt.bfloat16, kind="ExternalInput").ap()
    wvt_d = nc.dram_tensor("wvt", [H, GH], dt.bfloat16, kind="ExternalInput").ap()
    bqs_d = nc.dram_tensor("bqs", [P, G], dt.float32, kind="ExternalInput").ap()
    bk_d = nc.dram_tensor("bk", [P, G], dt.float32, kind="ExternalInput").ap()
    wot_d = nc.dram_tensor("wot", [GH, H], dt.bfloat16, kind="ExternalInput").ap()
    out_d = nc.dram_tensor("partial", [S, H], dt.bfloat16, kind="ExternalOutput").ap()

    xt_r = xt_d.rearrange("(ht p) s -> p ht s", p=P)      # [128, 16, 2048]
    wqt_r = wqt_d.rearrange("(ht p) o -> p ht o", p=P)    # [128, 16, 512]
    wkt_r = wkt_d.rearrange("(ht p) o -> p ht o", p=P)
    wvt_r = wvt_d.rearrange("(ht p) o -> p ht o", p=P)
    wot_r = wot_d.rearrange("(g p) o -> p g o", p=P)      # [128, 4, 2048]

    with tile.TileContext(nc) as tc:
        with (
            tc.tile_pool(name="consts", bufs=1) as consts,
            tc.tile_pool(name="wpool", bufs=1) as wpool,
            tc.tile_pool(name="xpool", bufs=2) as xpool,
            tc.tile_pool(name="big", bufs=1) as big,
            tc.tile_pool(name="epool", bufs=4) as epool,
            tc.tile_pool(name="small", bufs=2) as small,
            tc.tile_pool(name="psum", bufs=2, space="PSUM") as psum,
        ):
            # ---- HAM warm-up: get the PE clock gate to 8/8 before the
            # first real (DMA-gated) matmul arrives ----
            ones_sb = consts.tile([P, 512], dt.bfloat16)
            nc.vector.memset(ones_sb[:], 1.0)
            warm_ps = psum.tile([P, 2, QC], dt.float32, tag="st", name="ps")
            for _ in range(12):
                nc.tensor.matmul(warm_ps[:, 0, :], ones_sb[:, 0:P], ones_sb[:],
                                 start=True, stop=True)

            # ---- startup DMAs, critical-path first: the first q-matmul
            # (head 0) needs wq cols 0:128 and x chunk 0 ----
            wq_sb = epool.tile([P, HT, GH], dt.bfloat16, tag="e", name="wq_sb")
            nc.sync.dma_start(wq_sb[:, :, 0:HD], wqt_r[:, :, 0:HD])
            xt0_sb = xpool.tile([P, HT, XC], dt.bfloat16, tag="xt",
                                name="xt0_sb")
            for q4 in range(4):
                nc.sync.dma_start(xt0_sb[:, 4 * q4:4 * (q4 + 1), :],
                                  xt_r[:, 4 * q4:4 * (q4 + 1), 0:XC])
            bqs_sb = consts.tile([P, G], dt.float32)
            nc.sync.dma_start(bqs_sb[:], bqs_d)
            bk_sb = consts.tile([P, G], dt.float32)
            nc.sync.dma_start(bk_sb[:], bk_d)
            nc.sync.dma_start(wq_sb[:, :, HD:], wqt_r[:, :, HD:])
            xt1_sb = xpool.tile([P, HT, XC], dt.bfloat16, tag="xt",
                                name="xt1_sb")
            nc.sync.dma_start(xt1_sb[:], xt_r[:, :, XC:2 * XC])
            wk_sb = epool.tile([P, HT, GH], dt.bfloat16, tag="e", name="wk_sb")
            nc.sync.dma_start(wk_sb[:, :, 0:HD], wkt_r[:, :, 0:HD])
            nc.sync.dma_start(wk_sb[:, :, HD:], wkt_r[:, :, HD:])
            wv_sb = epool.tile([P, HT, GH], dt.bfloat16, tag="e", name="wv_sb")
            nc.sync.dma_start(wv_sb[:], wvt_r)

            qt_sb = big.tile([P, G, S], dt.bfloat16)   # q^T, scale+bias applied
            kt_sb = big.tile([P, G, S], dt.bfloat16)   # k^T, bias applied
            v_sb = big.tile([P, KT, GH], dt.bfloat16)  # v natural [s, o]
            at_sb = big.tile([P, G, S], dt.bfloat16)   # attn output^T

            # ---------------- phase-1 building blocks ----------------
            def q_group(xt_sb, xc, h):
                sl = slice(xc * XC, (xc + 1) * XC)
                psq = psum.tile([P, 512], dt.float32, tag="mm", name="psq")
                for ht in range(HT):
                    nc.tensor.matmul(psq,
                                     wq_sb[:, ht, h * HD:(h + 1) * HD],
                                     xt_sb[:, ht, :],
                                     start=(ht == 0), stop=(ht == HT - 1))
                nc.vector.tensor_scalar(qt_sb[:, h, sl], psq,
                                        SCALE, bqs_sb[:, h:h + 1],
                                        Alu.mult, Alu.add)

            def k_group(xt_sb, xc, h):
                sl = slice(xc * XC, (xc + 1) * XC)
                psk = psum.tile([P, 512], dt.float32, tag="mm", name="psk")
                for ht in range(HT):
                    nc.tensor.matmul(psk,
                                     wk_sb[:, ht, h * HD:(h + 1) * HD],
                                     xt_sb[:, ht, :],
                                     start=(ht == 0), stop=(ht == HT - 1))
                nc.vector.tensor_scalar_add(kt_sb[:, h, sl], psk,
                                            bk_sb[:, h:h + 1])

            def v_group(xt_sb, xc, sv):
                sm = xc * (XC // P) + sv
                psv = psum.tile([P, 512], dt.float32, tag="mm", name="psv")
                for ht in range(HT):
                    nc.tensor.matmul(psv,
                                     xt_sb[:, ht, sv * P:(sv + 1) * P],
                                     wv_sb[:, ht, :],
                                     start=(ht == 0), stop=(ht == HT - 1))
                nc.vector.tensor_copy(out=v_sb[:, sm, :], in_=psv)

            # ---------------- attention building blocks ----------------
            e_tiles = [None] * NCH
            pv_tiles = [None] * NCH
            zrow_tiles = [None] * NCH
            proj_q = []
            drain_flip = [0]

            def st_pair(idx, ms):
                # two 128-key score matmuls + one batched exp
                h, qc = idx % G, idx // G
                km = 2 * ms
                ps = psum.tile([P, 2, QC], dt.float32, tag="st", name="ps")
                for j in range(2):
                    nc.tensor.matmul(ps[:, j, :],
                                     kt_sb[:, h, (km + j) * P:(km + j + 1) * P],
                                     qt_sb[:, h, qc * QC:(qc + 1) * QC],
                                     start=True, stop=True)
                nc.scalar.activation(e_tiles[idx][:, km:km + 2, :], ps, Act.Exp)

            def pv_pair(idx, ms):
                h = idx % G
                pv, e = pv_tiles[idx], e_tiles[idx]
                for j in range(2):
                    km = 2 * ms + j
                    nc.tensor.matmul(pv, v_sb[:, km, h * HD:(h + 1) * HD],
                                     e[:, km, :],
                                     start=(km == 0), stop=(km == KT - 1))
                # level-1 of the denominator tree: fold this pair's odd tile
                # into the even one right after PV is done reading both
                nc.vector.tensor_add(e[:, 2 * ms, :], e[:, 2 * ms, :],
                                     e[:, 2 * ms + 1, :])
                if idx == NCH - 1 and ms % 2 == 1:
                    # last chunk: run the upper tree levels incrementally so
                    # the final zrow is ready right after the last PV pair
                    q4 = 2 * (ms - 1)
                    nc.vector.tensor_add(e[:, q4, :], e[:, q4, :],
                                         e[:, q4 + 2, :])
                    if ms == 3:
                        nc.vector.tensor_add(e[:, 0, :], e[:, 0, :],
                                             e[:, 4, :])
                    if ms == 7:
                        nc.vector.tensor_add(e[:, 8, :], e[:, 8, :],
                                             e[:, 12, :])
                        zr = small.tile([P, QC], dt.bfloat16, tag="zr",
                                        name="zr")
                        nc.vector.tensor_add(zr[:], e[:, 0, :], e[:, 8, :])
                        zrow_tiles[idx] = zr

            def tree_fin(idx):
                # levels 2-4 of the in-place pairwise bf16 tree; the final
                # sum lands in a small zrow buffer so the e-tile's last
                # reader is this tree, not the (later) ones-matmul
                e = e_tiles[idx]
                nc.vector.tensor_add(e[:, 0:KT:4, :], e[:, 0:KT:4, :],
                                     e[:, 2:KT:4, :])
                nc.vector.tensor_add(e[:, 0:KT:8, :], e[:, 0:KT:8, :],
                                     e[:, 4:KT:8, :])
                zr = small.tile([P, QC], dt.bfloat16, tag="zr", name="zr")
                nc.vector.tensor_add(zr[:], e[:, 0, :], e[:, 8, :])
                zrow_tiles[idx] = zr

            def zmm_norm(idx):
                # single ones-matmul: cross-partition sum of the tile-summed
                # exps, broadcast to all 128 partitions; then normalize
                h, qc = idx % G, idx // G
                z = psum.tile([P, QC], dt.float32, tag="mm", name="z")
                nc.tensor.matmul(z, ones_sb[:, 0:P], zrow_tiles[idx],
                                 start=True, stop=True)
                zi = small.tile([P, QC], dt.float32, tag="zi", name="zi")
                nc.vector.reciprocal_approx_fast(out=zi[:], in_=z)
                nc.vector.tensor_mul(out=at_sb[:, h, qc * QC:(qc + 1) * QC],
                                     in0=pv_tiles[idx], in1=zi[:])
                if h == G - 1:
                    for sv in range(QC // P):
                        for oc in range(H // 512):
                            proj_q.append((qc * (QC // P) + sv, oc))

            def proj_group(alt=False):
                if not proj_q:
                    return
                sm, oc = proj_q.pop(0)
                drain_flip[0] ^= 1
                pp = psum.tile([P, 512], dt.float32, tag="mm", name="pp")
                for g in range(G):
                    nc.tensor.matmul(pp,
                                     at_sb[:, g, sm * P:(sm + 1) * P],
                                     wo_sb[:, g, oc * 512:(oc + 1) * 512],
                                     start=(g == 0), stop=(g == G - 1))
                ob = small.tile([P, 512], dt.bfloat16, tag="ob", bufs=3,
                                name="ob")
                if alt and drain_flip[0]:
                    nc.scalar.copy(ob[:], pp)
                else:
                    nc.vector.tensor_copy(out=ob[:], in_=pp)
                nc.sync.dma_start(
                    out_d[sm * P:(sm + 1) * P, oc * 512:(oc + 1) * 512],
                    ob[:])

            # ---------------- Phase 1: QKV projections ----------------
            # xc0/xc1: q-projections only (need just wq + x), so the PE has
            # ~27us of work before the first k-group needs the wk DMA
            xts = [None] * NXC
            for xc in range(NXC):
                if xc == 0:
                    xts[xc] = xt0_sb
                elif xc == 1:
                    xts[xc] = xt1_sb
                else:
                    xts[xc] = xpool.tile([P, HT, XC], dt.bfloat16, tag="xt",
                                         name="xt_sb")
                    nc.sync.dma_start(xts[xc][:],
                                      xt_r[:, :, xc * XC:(xc + 1) * XC])
                if xc < 2:
                    for h in range(G):
                        q_group(xts[xc], xc, h)
                if xc == 1:
                    for x2 in range(2):
                        for h in range(G):
                            k_group(xts[x2], x2, h)
                    for x2 in range(2):
                        for sv in range(XC // P):
                            v_group(xts[x2], x2, sv)
                if xc == 2:
                    for h in range(G):
                        q_group(xts[xc], xc, h)
                    for h in range(G):
                        k_group(xts[xc], xc, h)
                    for sv in range(XC // P):
                        v_group(xts[xc], xc, sv)

            # last x chunk: k first, then weave the first two chunks' score
            # matmuls into the q/v groups so exp gets a head start
            xt3 = xts[NXC - 1]
            e_tiles[0] = epool.tile([P, KT, QC], dt.bfloat16, tag="e",
                                    name="e_sb")
            k_group(xt3, 3, 0)
            for h in range(1, G):
                k_group(xt3, 3, h)
                st_pair(0, 2 * (h - 1))
                st_pair(0, 2 * (h - 1) + 1)
            q_group(xt3, 3, 0)
            st_pair(0, 6)
            st_pair(0, 7)
            for h in range(1, G):
                q_group(xt3, 3, h)
            # wq now dead -> its pool slot is free for e_tiles[1]
            e_tiles[1] = epool.tile([P, KT, QC], dt.bfloat16, tag="e",
                                    name="e_sb")
            for sv in range(XC // P):
                v_group(xt3, 3, sv)
                st_pair(1, 2 * sv)
                st_pair(1, 2 * sv + 1)

            # out-proj weights: needed only from the first proj (~mid-kernel)
            wo_sb = wpool.tile([P, G, H], dt.bfloat16)
            nc.sync.dma_start(wo_sb[:], wot_r)

            # -------- Phase 2+3: mini-step interleaved chunk pipeline --------
            for j in range(2, 18):
                cur = j if j <= NCH - 1 else None
                pvi = j - 2 if j - 2 <= NCH - 1 else None
                zni = j - 3 if 0 <= j - 3 <= NCH - 1 else None
                if cur is not None:
                    e_tiles[cur] = epool.tile([P, KT, QC], dt.bfloat16,
                                              tag="e", name="e_sb")
                if pvi is not None:
                    pv_tiles[pvi] = psum.tile([P, QC], dt.float32, tag="pv",
                                              name="pv")
                for ms in range(8):
                    if cur is not None:
                        st_pair(cur, ms)
                    if pvi is not None:
                        pv_pair(pvi, ms)
                    if ms == 2 and zni is not None:
                        zmm_norm(zni)
                    if (ms % 2 == 1) and (cur is not None or ms < 7):
                        proj_group()
                if pvi is not None and pvi != NCH - 1:
                    tree_fin(pvi)
            proj_group(alt=True)
            proj_group(alt=True)
            zmm_norm(NCH - 1)
            while proj_q:
                proj_group(alt=True)

    nc.compile()
    return nc


def _get_nc():
    if "nc" not in _CACHE:
        _CACHE["nc"] = _build()
    return _CACHE["nc"]


def _make_in_maps(x, w_qkv, b_qkv, w_out):
    bf = ml_dtypes.bfloat16
    f32 = np.float32
    in_maps = []
    for c in range(N_CORES):
        b = c // 4
        g = c % 4
        lo = GH * g
        hi = GH * (g + 1)
        xt = np.ascontiguousarray(x[b].T).astype(bf)
        wqt = np.ascontiguousarray(w_qkv[lo:hi, :].T).astype(bf)
        wkt = np.ascontiguousarray(w_qkv[H + lo:H + hi, :].T).astype(bf)
        wvt = np.ascontiguousarray(w_qkv[2 * H + lo:2 * H + hi, :].T).astype(bf)
        bqs = np.ascontiguousarray(
            (b_qkv[lo:hi] * SCALE).astype(f32).reshape(G, P).T)
        bk = np.ascontiguousarray(
            b_qkv[H + lo:H + hi].astype(f32).reshape(G, P).T)
        wot = np.ascontiguousarray(w_out[:, lo:hi].T).astype(bf)
        in_maps.append({"xt": xt, "wqt": wqt, "wkt": wkt, "wvt": wvt,
                        "bqs": bqs, "bk": bk, "wot": wot})
    return in_maps


def kernel(x, w_qkv, b_qkv, w_out, b_out):
    import os
    import sys

    x = np.asarray(x, dtype=np.float32)
    w_qkv = np.asarray(w_qkv, dtype=np.float32)
    b_qkv = np.asarray(b_qkv, dtype=np.float32)
    w_out = np.asarray(w_out, dtype=np.float32)
    b_out = np.asarray(b_out, dtype=np.float32)

    from concourse.bass_utils import run_bass_kernel_spmd

    # NTFF tracing under axon needs the antenv.axon_hooks shim (test.py
    # installs it); without it a stray BASS_TRACE=1 in the environment would
    # crash the run — disable tracing in that case.
    if "antenv.axon_hooks" not in sys.modules:
        os.environ["BASS_NEVER_TRACE"] = "1"

    nc = _get_nc()
    in_maps = _make_in_maps(x, w_qkv, b_qkv, w_out)
    res = run_bass_kernel_spmd(nc, in_maps, core_ids=list(range(N_CORES)))
    _CACHE["last_results"] = res
    partials = [r["partial"] for r in res.results]

    bv = b_qkv[2 * H:3 * H]
    bias = b_out + w_out @ bv          # folded v-bias contribution
    out = np.empty((B, S, H), np.float32)
    for b in range(B):
        acc = partials[4 * b].astype(np.float32)
        for g in range(1, 4):
            acc += partials[4 * b + g].astype(np.float32)
        out[b] = acc + bias
    return out


# revision 16
# speedup vs baseline: 1.0176x; 1.0076x over previous
"""Trainium2 Bass kernel for NoTPAttention (dense transformer block:
fused QKV projection -> multi-head attention -> output projection).

Sharding (8 NeuronCores): core c handles batch b = c // 4 and the 4 heads
g = 4*(c % 4) .. 4*(c % 4)+3 (head-parallel tensor parallelism).  Each core
computes its heads' partial out-projection [S, H] in bf16; the host sums the
4 partials per batch in fp32 and adds the (folded) biases.

Numerics: all matmuls run in bf16 with fp32 PSUM accumulation.  Softmax is
computed without max-subtraction (scores are bounded, |s| < ~3.5) with the
normalization deferred to the attention *output*:
    attnT[d, q] = (sum_k v[k, d] * exp(sT[k, q])) / (sum_k exp(sT[k, q]))
The denominator: the 16 key-tiles of exp(sT) are pairwise-tree-summed on the
vector engine (4 strided in-place bf16 adds), then a single ones-matmul
broadcasts the cross-partition sum - 16x less tensor-engine work than a
full ones-matmul accumulation chain.  The v-bias is dropped in-kernel (it
contributes exactly b_v per row after normalization; the host folds
w_out @ b_v into the output bias).

Schedule: the PE instruction stream is emitted in "mini-steps" that weave
the score matmuls of chunk i between the PV matmuls of chunk i-2 and the
out-projection groups, so the in-order PE queue never stalls on the scalar
engine's exp pacing.  The first two chunks' score matmuls are woven into
the tail of the QKV phase (after their k/q slices complete) so the exp
stream gets a 2-chunk head start before PV consumption begins.  Startup:
8 warm-up matmuls on a memset ones tile lift the PE HAM clock gate to
8/8 before the first DMA-gated projection matmul issues.
"""

import numpy as np
import ml_dtypes

B, S, H = 2, 2048, 2048
NH, HD = 16, 128
P = 128
HT = H // P            # 16 hidden-dim tiles
G = 4                  # heads per core
GH = G * HD            # 512: head-group width per core
SCALE = 1.0 / float(np.sqrt(HD))
N_CORES = 8
XC = 512               # phase-1 x streaming chunk (s elements)
QC = 512               # attention query chunk
KT = S // P            # 16 key tiles
NXC = S // XC          # 4
NQC = S // QC          # 4
NCH = G * NQC          # 16 attention chunks

_CACHE = {}


def _build():
    import concourse.mybir as mybir
    import concourse.tile as tile
    from concourse import bacc

    dt = mybir.dt
    Alu = mybir.AluOpType
    Act = mybir.ActivationFunctionType

    nc = bacc.Bacc("TRN2", target_bir_lowering=False, debug=False,
                   enable_asserts=False)

    xt_d = nc.dram_tensor("xt", [H, S], dt.bfloat16, kind="ExternalInput").ap()
    wqt_d = nc.dram_tensor("wqt", [H, GH], dt.bfloat16, kind="ExternalInput").ap()
    wkt_d = nc.dram_tensor("wkt", [H, GH], dt.bfloat16, kind="ExternalInput").ap()
    wvt_d = nc.dram_tensor("wvt", [H, GH], dt.bfloat16, kind="ExternalInput").ap()
    bqs_d = nc.dram_tensor("bqs", [P, G], dt.float32, kind="ExternalInput").ap()
    bk_d = nc.dram_tensor("bk", [P, G], dt.float32, kind="ExternalInput").ap()
    wot_d = nc.dram_tensor("wot", [GH, H], dt.bfloat16, kind="ExternalInput").ap()
    out_d = nc.dram_tensor("partial", [S, H], dt.bfloat16, kind="ExternalOutput").ap()

    xt_r = xt_d.rearrange("(ht p) s -> p ht s", p=P)      # [128, 16, 2048]
    wqt_r = wqt_d.rearrange("(ht p) o -> p ht o", p=P)    # [128, 16, 512]
    wkt_r = wkt_d.rearrange("(ht p) o -> p ht o", p=P)
    wvt_r = wvt_d.rearrange("(ht p) o -> p ht o", p=P)
    wot_r = wot_d.rearrange("(g p) o -> p g o", p=P)      # [128, 4, 2048]

    with tile.TileContext(nc) as tc:
        with (
            tc.tile_pool(name="consts", bufs=1) as consts,
            tc.tile_pool(name="wpool", bufs=1) as wpool,
            tc.tile_pool(name="xpool", bufs=2) as xpool,
            tc.tile_pool(name="big", bufs=1) as big,
            tc.tile_pool(name="epool", bufs=4) as epool,
            tc.tile_pool(name="small", bufs=2) as small,
            tc.tile_pool(name="psum", bufs=2, space="PSUM") as psum,
        ):
            # ---- HAM warm-up: get the PE clock gate to 8/8 before the
            # first real (DMA-gated) matmul arrives ----
            ones_sb = consts.tile([P, 512], dt.bfloat16)
            nc.vector.memset(ones_sb[:], 1.0)
            warm_ps = psum.tile([P, 2, QC], dt.float32, tag="st", name="ps")
            for _ in range(12):
                nc.tensor.matmul(warm_ps[:, 0, :], ones_sb[:, 0:P], ones_sb[:],
                                 start=True, stop=True)

            # ---- startup DMAs, critical-path first: the first q-matmul
            # (head 0) needs wq cols 0:128 and x chunk 0 ----
            wq_sb = epool.tile([P, HT, GH], dt.bfloat16, tag="e", name="wq_sb")
            xt0_sb = xpool.tile([P, HT, XC], dt.bfloat16, tag="xt",
                                name="xt0_sb")
            xt1_sb = xpool.tile([P, HT, XC], dt.bfloat16, tag="xt",
                                name="xt1_sb")
            # fine-grained interleave so the q-matmuls chase each transfer:
            # wq head-slice h feeds q(h); x quads feed the ht accumulation
            nc.sync.dma_start(wq_sb[:, :, 0:HD], wqt_r[:, :, 0:HD])
            bqs_sb = consts.tile([P, G], dt.float32)
            nc.sync.dma_start(bqs_sb[:], bqs_d)
            bk_sb = consts.tile([P, G], dt.float32)
            nc.sync.dma_start(bk_sb[:], bk_d)
            nc.sync.dma_start(xt0_sb[:, 0:4, :], xt_r[:, 0:4, 0:XC])
            nc.sync.dma_start(xt0_sb[:, 4:8, :], xt_r[:, 4:8, 0:XC])
            nc.sync.dma_start(wq_sb[:, :, HD:2 * HD], wqt_r[:, :, HD:2 * HD])
            nc.sync.dma_start(xt0_sb[:, 8:12, :], xt_r[:, 8:12, 0:XC])
            nc.sync.dma_start(wq_sb[:, :, 2 * HD:3 * HD],
                              wqt_r[:, :, 2 * HD:3 * HD])
            nc.sync.dma_start(xt0_sb[:, 12:16, :], xt_r[:, 12:16, 0:XC])
            nc.sync.dma_start(wq_sb[:, :, 3 * HD:], wqt_r[:, :, 3 * HD:])
            for q4 in range(4):
                nc.sync.dma_start(xt1_sb[:, 4 * q4:4 * (q4 + 1), :],
                                  xt_r[:, 4 * q4:4 * (q4 + 1), XC:2 * XC])
            wk_sb = epool.tile([P, HT, GH], dt.bfloat16, tag="e", name="wk_sb")
            nc.sync.dma_start(wk_sb[:, :, 0:HD], wkt_r[:, :, 0:HD])
            nc.sync.dma_start(wk_sb[:, :, HD:], wkt_r[:, :, HD:])
            wv_sb = epool.tile([P, HT, GH], dt.bfloat16, tag="e", name="wv_sb")
            nc.sync.dma_start(wv_sb[:], wvt_r)

            qt_sb = big.tile([P, G, S], dt.bfloat16)   # q^T, scale+bias applied
            kt_sb = big.tile([P, G, S], dt.bfloat16)   # k^T, bias applied
            v_sb = big.tile([P, KT, GH], dt.bfloat16)  # v natural [s, o]
            at_sb = big.tile([P, G, S], dt.bfloat16)   # attn output^T

            # ---------------- phase-1 building blocks ----------------
            def q_group(xt_sb, xc, h):
                sl = slice(xc * XC, (xc + 1) * XC)
                psq = psum.tile([P, 512], dt.float32, tag="mm", name="psq")
                for ht in range(HT):
                    nc.tensor.matmul(psq,
                                     wq_sb[:, ht, h * HD:(h + 1) * HD],
                                     xt_sb[:, ht, :],
                                     start=(ht == 0), stop=(ht == HT - 1))
                nc.vector.tensor_scalar(qt_sb[:, h, sl], psq,
                                        SCALE, bqs_sb[:, h:h + 1],
                                        Alu.mult, Alu.add)

            def k_group(xt_sb, xc, h):
                sl = slice(xc * XC, (xc + 1) * XC)
                psk = psum.tile([P, 512], dt.float32, tag="mm", name="psk")
                for ht in range(HT):
                    nc.tensor.matmul(psk,
                                     wk_sb[:, ht, h * HD:(h + 1) * HD],
                                     xt_sb[:, ht, :],
                                     start=(ht == 0), stop=(ht == HT - 1))
                nc.vector.tensor_scalar_add(kt_sb[:, h, sl], psk,
                                            bk_sb[:, h:h + 1])

            def v_group(xt_sb, xc, sv):
                sm = xc * (XC // P) + sv
                psv = psum.tile([P, 512], dt.float32, tag="mm", name="psv")
                for ht in range(HT):
                    nc.tensor.matmul(psv,
                                     xt_sb[:, ht, sv * P:(sv + 1) * P],
                                     wv_sb[:, ht, :],
                                     start=(ht == 0), stop=(ht == HT - 1))
                nc.vector.tensor_copy(out=v_sb[:, sm, :], in_=psv)

            # ---------------- attention building blocks ----------------
            e_tiles = [None] * NCH
            pv_tiles = [None] * NCH
            zrow_tiles = [None] * NCH
            zch_tiles = [None] * NCH
            proj_q = []
            drain_flip = [0]

            def st_pair(idx, ms):
                # two 128-key score matmuls + one batched exp
                h, qc = idx % G, idx // G
                km = 2 * ms
                ps = psum.tile([P, 2, QC], dt.float32, tag="st", name="ps")
                for j in range(2):
                    nc.tensor.matmul(ps[:, j, :],
                                     kt_sb[:, h, (km + j) * P:(km + j + 1) * P],
                                     qt_sb[:, h, qc * QC:(qc + 1) * QC],
                                     start=True, stop=True)
                nc.scalar.activation(e_tiles[idx][:, km:km + 2, :], ps, Act.Exp)

            def pv_pair(idx, ms):
                h = idx % G
                pv, e = pv_tiles[idx], e_tiles[idx]
                for j in range(2):
                    km = 2 * ms + j
                    nc.tensor.matmul(pv, v_sb[:, km, h * HD:(h + 1) * HD],
                                     e[:, km, :],
                                     start=(km == 0), stop=(km == KT - 1))
                if idx >= NCH - 2:
                    # flush chunks: the PE has slack and the DVE is the
                    # flush bottleneck - accumulate the denominator with a
                    # ones-matmul chain into a free st-tag PSUM bank
                    for j in range(2):
                        km = 2 * ms + j
                        nc.tensor.matmul(zch_tiles[idx], ones_sb[:, 0:P],
                                         e[:, km, :],
                                         start=(km == 0), stop=(km == KT - 1))
                else:
                    # level-1 of the denominator tree: fold this pair's odd
                    # tile into the even one after PV is done reading both
                    nc.vector.tensor_add(e[:, 2 * ms, :], e[:, 2 * ms, :],
                                         e[:, 2 * ms + 1, :])

            def tree_fin(idx):
                # levels 2-4 of the in-place pairwise bf16 tree; the final
                # sum lands in a small zrow buffer so the e-tile's last
                # reader is this tree, not the (later) ones-matmul
                e = e_tiles[idx]
                nc.vector.tensor_add(e[:, 0:KT:4, :], e[:, 0:KT:4, :],
                                     e[:, 2:KT:4, :])
                nc.vector.tensor_add(e[:, 0:KT:8, :], e[:, 0:KT:8, :],
                                     e[:, 4:KT:8, :])
                zr = small.tile([P, QC], dt.bfloat16, tag="zr", name="zr")
                nc.vector.tensor_add(zr[:], e[:, 0, :], e[:, 8, :])
                zrow_tiles[idx] = zr

            def zmm_norm(idx):
                # single ones-matmul: cross-partition sum of the tile-summed
                # exps, broadcast to all 128 partitions; then normalize
                h, qc = idx % G, idx // G
                if zch_tiles[idx] is not None:
                    z = zch_tiles[idx]
                else:
                    z = psum.tile([P, QC], dt.float32, tag="mm", name="z")
                    nc.tensor.matmul(z, ones_sb[:, 0:P], zrow_tiles[idx],
                                     start=True, stop=True)
                zi = small.tile([P, QC], dt.float32, tag="zi", name="zi")
                nc.vector.reciprocal_approx_fast(out=zi[:], in_=z)
                nc.vector.tensor_mul(out=at_sb[:, h, qc * QC:(qc + 1) * QC],
                                     in0=pv_tiles[idx], in1=zi[:])
                if h == G - 1:
                    for sv in range(QC // P):
                        for oc in range(H // 512):
                            proj_q.append((qc * (QC // P) + sv, oc))

            def proj_group(alt=False):
                if not proj_q:
                    return
                sm, oc = proj_q.pop(0)
                drain_flip[0] ^= 1
                pp = psum.tile([P, 512], dt.float32, tag="mm", name="pp")
                for g in range(G):
                    nc.tensor.matmul(pp,
                                     at_sb[:, g, sm * P:(sm + 1) * P],
                                     wo_sb[:, g, oc * 512:(oc + 1) * 512],
                                     start=(g == 0), stop=(g == G - 1))
                ob = small.tile([P, 512], dt.bfloat16, tag="ob", bufs=3,
                                name="ob")
                if alt and drain_flip[0]:
                    nc.scalar.copy(ob[:], pp)
                else:
                    nc.vector.tensor_copy(out=ob[:], in_=pp)
                nc.sync.dma_start(
                    out_d[sm * P:(sm + 1) * P, oc * 512:(oc + 1) * 512],
                    ob[:])

            # ---------------- Phase 1: QKV projections ----------------
            # xc0/xc1: q-projections only (need just wq + x), so the PE has
            # ~27us of work before the first k-group needs the wk DMA
            xts = [None] * NXC
            for xc in range(NXC):
                if xc == 0:
                    xts[xc] = xt0_sb
                elif xc == 1:
                    xts[xc] = xt1_sb
                else:
                    xts[xc] = xpool.tile([P, HT, XC], dt.bfloat16, tag="xt",
                                         name="xt_sb")
                    nc.sync.dma_start(xts[xc][:],
                                      xt_r[:, :, xc * XC:(xc + 1) * XC])
                if xc < 2:
                    for h in range(G):
                        q_group(xts[xc], xc, h)
                if xc == 1:
                    for x2 in range(2):
                        for h in range(G):
                            k_group(xts[x2], x2, h)
                    for x2 in range(2):
                        for sv in range(XC // P):
                            v_group(xts[x2], x2, sv)
                if xc == 2:
                    for h in range(G):
                        q_group(xts[xc], xc, h)
                    for h in range(G):
                        k_group(xts[xc], xc, h)
                    for sv in range(XC // P):
                        v_group(xts[xc], xc, sv)

            # last x chunk: k first, then weave the first two chunks' score
            # matmuls into the q/v groups so exp gets a head start
            xt3 = xts[NXC - 1]
            e_tiles[0] = epool.tile([P, KT, QC], dt.bfloat16, tag="e",
                                    name="e_sb")
            k_group(xt3, 3, 0)
            for h in range(1, G):
                k_group(xt3, 3, h)
                st_pair(0, 2 * (h - 1))
                st_pair(0, 2 * (h - 1) + 1)
            q_group(xt3, 3, 0)
            st_pair(0, 6)
            st_pair(0, 7)
            for h in range(1, G):
                q_group(xt3, 3, h)
            # wq now dead -> its pool slot is free for e_tiles[1]
            e_tiles[1] = epool.tile([P, KT, QC], dt.bfloat16, tag="e",
                                    name="e_sb")
            for sv in range(XC // P):
                v_group(xt3, 3, sv)
                st_pair(1, 2 * sv)
                st_pair(1, 2 * sv + 1)

            # out-proj weights: needed only from the first proj (~mid-kernel)
            wo_sb = wpool.tile([P, G, H], dt.bfloat16)
            nc.sync.dma_start(wo_sb[:], wot_r)

            # -------- Phase 2+3: mini-step interleaved chunk pipeline --------
            for j in range(2, 18):
                cur = j if j <= NCH - 1 else None
                pvi = j - 2 if j - 2 <= NCH - 1 else None
                zni = j - 3 if 0 <= j - 3 <= NCH - 1 else None
                if cur is not None:
                    e_tiles[cur] = epool.tile([P, KT, QC], dt.bfloat16,
                                              tag="e", name="e_sb")
                if pvi is not None:
                    pv_tiles[pvi] = psum.tile([P, QC], dt.float32, tag="pv",
                                              name="pv")
                    if pvi >= NCH - 2:
                        zt = psum.tile([P, 2, QC], dt.float32, tag="st",
                                       name="zt")
                        zch_tiles[pvi] = zt[:, 0, :]
                for ms in range(8):
                    if cur is not None:
                        st_pair(cur, ms)
                    if pvi is not None:
                        pv_pair(pvi, ms)
                    if ms == 2 and zni is not None:
                        zmm_norm(zni)
                    if (ms % 2 == 1) and (cur is not None or ms < 7):
                        proj_group(alt=(cur is None))
                if pvi is not None and pvi < NCH - 2:
                    tree_fin(pvi)
            proj_group(alt=True)
            proj_group(alt=True)
            zmm_norm(NCH - 1)
            while proj_q:
                proj_group(alt=True)

    nc.compile()
    return nc


def _get_nc():
    if "nc" not in _CACHE:
        _CACHE["nc"] = _build()
    return _CACHE["nc"]


def _make_in_maps(x, w_qkv, b_qkv, w_out):
    bf = ml_dtypes.bfloat16
    f32 = np.float32
    in_maps = []
    for c in range(N_CORES):
        b = c // 4
        g = c % 4
        lo = GH * g
        hi = GH * (g + 1)
        xt = np.ascontiguousarray(x[b].T).astype(bf)
        wqt = np.ascontiguousarray(w_qkv[lo:hi, :].T).astype(bf)
        wkt = np.ascontiguousarray(w_qkv[H + lo:H + hi, :].T).astype(bf)
        wvt = np.ascontiguousarray(w_qkv[2 * H + lo:2 * H + hi, :].T).astype(bf)
        bqs = np.ascontiguousarray(
            (b_qkv[lo:hi] * SCALE).astype(f32).reshape(G, P).T)
        bk = np.ascontiguousarray(
            b_qkv[H + lo:H + hi].astype(f32).reshape(G, P).T)
        wot = np.ascontiguousarray(w_out[:, lo:hi].T).astype(bf)
        in_maps.append({"xt": xt, "wqt": wqt, "wkt": wkt, "wvt": wvt,
                        "bqs": bqs, "bk": bk, "wot": wot})
    return in_maps


def kernel(x, w_qkv, b_qkv, w_out, b_out):
    import os
    import sys

    x = np.asarray(x, dtype=np.float32)
    w_qkv = np.asarray(w_qkv, dtype=np.float32)
    b_qkv = np.asarray(b_qkv, dtype=np.float32)
    w_out = np.asarray(w_out, dtype=np.float32)
    b_out = np.asarray(b_out, dtype=np.float32)

    from concourse.bass_utils import run_bass_kernel_spmd

    # NTFF tracing under axon needs the antenv.axon_hooks shim (test.py
    # installs it); without it a stray BASS_TRACE=1 in the environment would
    # crash the run — disable tracing in that case.
    if "antenv.axon_hooks" not in sys.modules:
        os.environ["BASS_NEVER_TRACE"] = "1"

    nc = _get_nc()
    in_maps = _make_in_maps(x, w_qkv, b_qkv, w_out)
    res = run_bass_kernel_spmd(nc, in_maps, core_ids=list(range(N_CORES)))
    _CACHE["last_results"] = res
    partials = [r["partial"] for r in res.results]

    bv = b_qkv[2 * H:3 * H]
    bias = b_out + w_out @ bv          # folded v-bias contribution
    out = np.empty((B, S, H), np.float32)
    for b in range(B):
        acc = partials[4 * b].astype(np.float32)
        for g in range(1, 4):
            acc += partials[4 * b + g].astype(np.float32)
        out[b] = acc + bias
    return out


# revision 17
# speedup vs baseline: 1.0247x; 1.0069x over previous
"""Trainium2 Bass kernel for NoTPAttention (dense transformer block:
fused QKV projection -> multi-head attention -> output projection).

Sharding (8 NeuronCores): core c handles batch b = c // 4 and the 4 heads
g = 4*(c % 4) .. 4*(c % 4)+3 (head-parallel tensor parallelism).  Each core
computes its heads' partial out-projection [S, H] in bf16; the host sums the
4 partials per batch in fp32 and adds the (folded) biases.

Numerics: all matmuls run in bf16 with fp32 PSUM accumulation.  Softmax is
computed without max-subtraction (scores are bounded, |s| < ~3.5) with the
normalization deferred to the attention *output*:
    attnT[d, q] = (sum_k v[k, d] * exp(sT[k, q])) / (sum_k exp(sT[k, q]))
The denominator: the 16 key-tiles of exp(sT) are pairwise-tree-summed on the
vector engine (4 strided in-place bf16 adds), then a single ones-matmul
broadcasts the cross-partition sum - 16x less tensor-engine work than a
full ones-matmul accumulation chain.  The v-bias is dropped in-kernel (it
contributes exactly b_v per row after normalization; the host folds
w_out @ b_v into the output bias).

Schedule: the PE instruction stream is emitted in "mini-steps" that weave
the score matmuls of chunk i between the PV matmuls of chunk i-2 and the
out-projection groups, so the in-order PE queue never stalls on the scalar
engine's exp pacing.  The first two chunks' score matmuls are woven into
the tail of the QKV phase (after their k/q slices complete) so the exp
stream gets a 2-chunk head start before PV consumption begins.  Startup:
8 warm-up matmuls on a memset ones tile lift the PE HAM clock gate to
8/8 before the first DMA-gated projection matmul issues.
"""

import numpy as np
import ml_dtypes

B, S, H = 2, 2048, 2048
NH, HD = 16, 128
P = 128
HT = H // P            # 16 hidden-dim tiles
G = 4                  # heads per core
GH = G * HD            # 512: head-group width per core
SCALE = 1.0 / float(np.sqrt(HD))
N_CORES = 8
XC = 512               # phase-1 x streaming chunk (s elements)
QC = 512               # attention query chunk
KT = S // P            # 16 key tiles
NXC = S // XC          # 4
NQC = S // QC          # 4
NCH = G * NQC          # 16 attention chunks

_CACHE = {}


def _build():
    import concourse.mybir as mybir
    import concourse.tile as tile
    from concourse import bacc

    dt = mybir.dt
    Alu = mybir.AluOpType
    Act = mybir.ActivationFunctionType

    nc = bacc.Bacc("TRN2", target_bir_lowering=False, debug=False,
                   enable_asserts=False)

    xt_d = nc.dram_tensor("xt", [H, S], dt.bfloat16, kind="ExternalInput").ap()
    wqt_d = nc.dram_tensor("wqt", [H, GH], dt.bfloat16, kind="ExternalInput").ap()
    wkt_d = nc.dram_tensor("wkt", [H, GH], dt.bfloat16, kind="ExternalInput").ap()
    wvt_d = nc.dram_tensor("wvt", [H, GH], dt.bfloat16, kind="ExternalInput").ap()
    bqs_d = nc.dram_tensor("bqs", [P, G], dt.float32, kind="ExternalInput").ap()
    bk_d = nc.dram_tensor("bk", [P, G], dt.float32, kind="ExternalInput").ap()
    wot_d = nc.dram_tensor("wot", [GH, H], dt.bfloat16, kind="ExternalInput").ap()
    out_d = nc.dram_tensor("partial", [S, H], dt.bfloat16, kind="ExternalOutput").ap()

    xt_r = xt_d.rearrange("(ht p) s -> p ht s", p=P)      # [128, 16, 2048]
    wqt_r = wqt_d.rearrange("(ht p) o -> p ht o", p=P)    # [128, 16, 512]
    wkt_r = wkt_d.rearrange("(ht p) o -> p ht o", p=P)
    wvt_r = wvt_d.rearrange("(ht p) o -> p ht o", p=P)
    wot_r = wot_d.rearrange("(g p) o -> p g o", p=P)      # [128, 4, 2048]

    with tile.TileContext(nc) as tc:
        with (
            tc.tile_pool(name="consts", bufs=1) as consts,
            tc.tile_pool(name="wpool", bufs=1) as wpool,
            tc.tile_pool(name="xpool", bufs=2) as xpool,
            tc.tile_pool(name="big", bufs=1) as big,
            tc.tile_pool(name="epool", bufs=4) as epool,
            tc.tile_pool(name="small", bufs=2) as small,
            tc.tile_pool(name="psum", bufs=2, space="PSUM") as psum,
        ):
            # ---- HAM warm-up: get the PE clock gate to 8/8 before the
            # first real (DMA-gated) matmul arrives ----
            ones_sb = consts.tile([P, 512], dt.bfloat16)
            nc.vector.memset(ones_sb[:], 1.0)
            warm_ps = psum.tile([P, 2, QC], dt.float32, tag="st", name="ps")
            for _ in range(8):
                nc.tensor.matmul(warm_ps[:, 0, :], ones_sb[:, 0:P], ones_sb[:],
                                 start=True, stop=True)

            # ---- startup DMAs, critical-path first: the first q-matmul
            # (head 0) needs wq cols 0:128 and x chunk 0 ----
            wq_sb = epool.tile([P, HT, GH], dt.bfloat16, tag="e", name="wq_sb")
            xt0_sb = xpool.tile([P, HT, XC], dt.bfloat16, tag="xt",
                                name="xt0_sb")
            xt1_sb = xpool.tile([P, HT, XC], dt.bfloat16, tag="xt",
                                name="xt1_sb")
            # fine-grained interleave so the q-matmuls chase each transfer:
            # wq head-slice h feeds q(h); x quads feed the ht accumulation
            nc.sync.dma_start(wq_sb[:, :, 0:HD], wqt_r[:, :, 0:HD])
            bqs_sb = consts.tile([P, G], dt.float32)
            nc.sync.dma_start(bqs_sb[:], bqs_d)
            bk_sb = consts.tile([P, G], dt.float32)
            nc.sync.dma_start(bk_sb[:], bk_d)
            for q4 in range(4):
                nc.sync.dma_start(xt0_sb[:, 4 * q4:4 * (q4 + 1), :],
                                  xt_r[:, 4 * q4:4 * (q4 + 1), 0:XC])
            nc.sync.dma_start(wq_sb[:, :, HD:2 * HD], wqt_r[:, :, HD:2 * HD])
            nc.sync.dma_start(wq_sb[:, :, 2 * HD:3 * HD],
                              wqt_r[:, :, 2 * HD:3 * HD])
            nc.sync.dma_start(wq_sb[:, :, 3 * HD:], wqt_r[:, :, 3 * HD:])
            for q4 in range(4):
                nc.sync.dma_start(xt1_sb[:, 4 * q4:4 * (q4 + 1), :],
                                  xt_r[:, 4 * q4:4 * (q4 + 1), XC:2 * XC])
            wk_sb = epool.tile([P, HT, GH], dt.bfloat16, tag="e", name="wk_sb")
            nc.sync.dma_start(wk_sb[:, :, 0:HD], wkt_r[:, :, 0:HD])
            nc.sync.dma_start(wk_sb[:, :, HD:], wkt_r[:, :, HD:])
            wv_sb = epool.tile([P, HT, GH], dt.bfloat16, tag="e", name="wv_sb")
            nc.sync.dma_start(wv_sb[:], wvt_r)

            qt_sb = big.tile([P, G, S], dt.bfloat16)   # q^T, scale+bias applied
            kt_sb = big.tile([P, G, S], dt.bfloat16)   # k^T, bias applied
            v_sb = big.tile([P, KT, GH], dt.bfloat16)  # v natural [s, o]
            at_sb = big.tile([P, G, S], dt.bfloat16)   # attn output^T

            # ---------------- phase-1 building blocks ----------------
            def q_group(xt_sb, xc, h):
                sl = slice(xc * XC, (xc + 1) * XC)
                psq = psum.tile([P, 512], dt.float32, tag="mm", name="psq")
                for ht in range(HT):
                    nc.tensor.matmul(psq,
                                     wq_sb[:, ht, h * HD:(h + 1) * HD],
                                     xt_sb[:, ht, :],
                                     start=(ht == 0), stop=(ht == HT - 1))
                nc.vector.tensor_scalar(qt_sb[:, h, sl], psq,
                                        SCALE, bqs_sb[:, h:h + 1],
                                        Alu.mult, Alu.add)

            def k_group(xt_sb, xc, h):
                sl = slice(xc * XC, (xc + 1) * XC)
                psk = psum.tile([P, 512], dt.float32, tag="mm", name="psk")
                for ht in range(HT):
                    nc.tensor.matmul(psk,
                                     wk_sb[:, ht, h * HD:(h + 1) * HD],
                                     xt_sb[:, ht, :],
                                     start=(ht == 0), stop=(ht == HT - 1))
                nc.vector.tensor_scalar_add(kt_sb[:, h, sl], psk,
                                            bk_sb[:, h:h + 1])

            def v_group(xt_sb, xc, sv):
                sm = xc * (XC // P) + sv
                psv = psum.tile([P, 512], dt.float32, tag="mm", name="psv")
                for ht in range(HT):
                    nc.tensor.matmul(psv,
                                     xt_sb[:, ht, sv * P:(sv + 1) * P],
                                     wv_sb[:, ht, :],
                                     start=(ht == 0), stop=(ht == HT - 1))
                nc.vector.tensor_copy(out=v_sb[:, sm, :], in_=psv)

            # ---------------- attention building blocks ----------------
            e_tiles = [None] * NCH
            pv_tiles = [None] * NCH
            zrow_tiles = [None] * NCH
            zch_tiles = [None] * NCH
            proj_q = []
            drain_flip = [0]

            def st_pair(idx, ms):
                # two 128-key score matmuls + one batched exp
                h, qc = idx % G, idx // G
                km = 2 * ms
                ps = psum.tile([P, 2, QC], dt.float32, tag="st", name="ps")
                for j in range(2):
                    nc.tensor.matmul(ps[:, j, :],
                                     kt_sb[:, h, (km + j) * P:(km + j + 1) * P],
                                     qt_sb[:, h, qc * QC:(qc + 1) * QC],
                                     start=True, stop=True)
                nc.scalar.activation(e_tiles[idx][:, km:km + 2, :], ps, Act.Exp)

            def pv_pair(idx, ms):
                h = idx % G
                pv, e = pv_tiles[idx], e_tiles[idx]
                for j in range(2):
                    km = 2 * ms + j
                    nc.tensor.matmul(pv, v_sb[:, km, h * HD:(h + 1) * HD],
                                     e[:, km, :],
                                     start=(km == 0), stop=(km == KT - 1))
                if idx >= NCH - 2:
                    # flush chunks: the PE has slack and the DVE is the
                    # flush bottleneck - accumulate the denominator with a
                    # ones-matmul chain into a free st-tag PSUM bank
                    for j in range(2):
                        km = 2 * ms + j
                        nc.tensor.matmul(zch_tiles[idx], ones_sb[:, 0:P],
                                         e[:, km, :],
                                         start=(km == 0), stop=(km == KT - 1))
                else:
                    # level-1 of the denominator tree: fold this pair's odd
                    # tile into the even one after PV is done reading both
                    nc.vector.tensor_add(e[:, 2 * ms, :], e[:, 2 * ms, :],
                                         e[:, 2 * ms + 1, :])

            def tree_fin(idx):
                # levels 2-4 of the in-place pairwise bf16 tree; the final
                # sum lands in a small zrow buffer so the e-tile's last
                # reader is this tree, not the (later) ones-matmul
                e = e_tiles[idx]
                nc.vector.tensor_add(e[:, 0:KT:4, :], e[:, 0:KT:4, :],
                                     e[:, 2:KT:4, :])
                nc.vector.tensor_add(e[:, 0:KT:8, :], e[:, 0:KT:8, :],
                                     e[:, 4:KT:8, :])
                zr = small.tile([P, QC], dt.bfloat16, tag="zr", name="zr")
                nc.vector.tensor_add(zr[:], e[:, 0, :], e[:, 8, :])
                zrow_tiles[idx] = zr

            def zmm_norm(idx):
                # single ones-matmul: cross-partition sum of the tile-summed
                # exps, broadcast to all 128 partitions; then normalize
                h, qc = idx % G, idx // G
                if zch_tiles[idx] is not None:
                    z = zch_tiles[idx]
                else:
                    z = psum.tile([P, QC], dt.float32, tag="mm", name="z")
                    nc.tensor.matmul(z, ones_sb[:, 0:P], zrow_tiles[idx],
                                     start=True, stop=True)
                zi = small.tile([P, QC], dt.float32, tag="zi", name="zi")
                nc.vector.reciprocal_approx_fast(out=zi[:], in_=z)
                nc.vector.tensor_mul(out=at_sb[:, h, qc * QC:(qc + 1) * QC],
                                     in0=pv_tiles[idx], in1=zi[:])
                if h == G - 1:
                    for sv in range(QC // P):
                        for oc in range(H // 512):
                            proj_q.append((qc * (QC // P) + sv, oc))

            def proj_group(alt=False):
                if not proj_q:
                    return
                sm, oc = proj_q.pop(0)
                drain_flip[0] ^= 1
                pp = psum.tile([P, 512], dt.float32, tag="mm", name="pp")
                for g in range(G):
                    nc.tensor.matmul(pp,
                                     at_sb[:, g, sm * P:(sm + 1) * P],
                                     wo_sb[:, g, oc * 512:(oc + 1) * 512],
                                     start=(g == 0), stop=(g == G - 1))
                ob = small.tile([P, 512], dt.bfloat16, tag="ob", bufs=4,
                                name="ob")
                if alt and drain_flip[0]:
                    nc.scalar.copy(ob[:], pp)
                else:
                    nc.vector.tensor_copy(out=ob[:], in_=pp)
                nc.sync.dma_start(
                    out_d[sm * P:(sm + 1) * P, oc * 512:(oc + 1) * 512],
                    ob[:])

            # ---------------- Phase 1: QKV projections ----------------
            # xc0/xc1: q-projections only (need just wq + x), so the PE has
            # ~27us of work before the first k-group needs the wk DMA
            xts = [None] * NXC
            for xc in range(NXC):
                if xc == 0:
                    xts[xc] = xt0_sb
                elif xc == 1:
                    xts[xc] = xt1_sb
                else:
                    xts[xc] = xpool.tile([P, HT, XC], dt.bfloat16, tag="xt",
                                         name="xt_sb")
                    nc.sync.dma_start(xts[xc][:],
                                      xt_r[:, :, xc * XC:(xc + 1) * XC])
                if xc < 2:
                    for h in range(G):
                        q_group(xts[xc], xc, h)
                if xc == 1:
                    for x2 in range(2):
                        for h in range(G):
                            k_group(xts[x2], x2, h)
                    for x2 in range(2):
                        for sv in range(XC // P):
                            v_group(xts[x2], x2, sv)
                if xc == 2:
                    for h in range(G):
                        q_group(xts[xc], xc, h)
                    for h in range(G):
                        k_group(xts[xc], xc, h)
                    for sv in range(XC // P):
                        v_group(xts[xc], xc, sv)

            # last x chunk: k first, then weave the first two chunks' score
            # matmuls into the q/v groups so exp gets a head start
            xt3 = xts[NXC - 1]
            e_tiles[0] = epool.tile([P, KT, QC], dt.bfloat16, tag="e",
                                    name="e_sb")
            k_group(xt3, 3, 0)
            for h in range(1, G):
                k_group(xt3, 3, h)
                st_pair(0, 2 * (h - 1))
                st_pair(0, 2 * (h - 1) + 1)
            q_group(xt3, 3, 0)
            st_pair(0, 6)
            st_pair(0, 7)
            for h in range(1, G):
                q_group(xt3, 3, h)
            # wq now dead -> its pool slot is free for e_tiles[1]
            e_tiles[1] = epool.tile([P, KT, QC], dt.bfloat16, tag="e",
                                    name="e_sb")
            for sv in range(XC // P):
                v_group(xt3, 3, sv)
                st_pair(1, 2 * sv)
                st_pair(1, 2 * sv + 1)

            # out-proj weights: needed only from the first proj (~mid-kernel)
            wo_sb = wpool.tile([P, G, H], dt.bfloat16)
            nc.sync.dma_start(wo_sb[:], wot_r)

            # -------- Phase 2+3: mini-step interleaved chunk pipeline --------
            for j in range(2, 18):
                cur = j if j <= NCH - 1 else None
                pvi = j - 2 if j - 2 <= NCH - 1 else None
                zni = j - 3 if 0 <= j - 3 <= NCH - 1 else None
                if cur is not None:
                    e_tiles[cur] = epool.tile([P, KT, QC], dt.bfloat16,
                                              tag="e", name="e_sb")
                if pvi is not None:
                    pv_tiles[pvi] = psum.tile([P, QC], dt.float32, tag="pv",
                                              name="pv")
                    if pvi >= NCH - 2:
                        zt = psum.tile([P, 2, QC], dt.float32, tag="st",
                                       name="zt")
                        zch_tiles[pvi] = zt[:, 0, :]
                for ms in range(8):
                    if cur is not None:
                        st_pair(cur, ms)
                    if pvi is not None:
                        pv_pair(pvi, ms)
                    if ms == 2 and zni is not None:
                        zmm_norm(zni)
                    if ms % 2 == 1:
                        proj_group(alt=(cur is None))
                if pvi is not None and pvi < NCH - 2:
                    tree_fin(pvi)
            proj_group(alt=True)
            proj_group(alt=True)
            zmm_norm(NCH - 1)
            while proj_q:
                proj_group(alt=True)

    nc.compile()
    return nc


def _get_nc():
    if "nc" not in _CACHE:
        _CACHE["nc"] = _build()
    return _CACHE["nc"]


def _make_in_maps(x, w_qkv, b_qkv, w_out):
    bf = ml_dtypes.bfloat16
    f32 = np.float32
    in_maps = []
    for c in range(N_CORES):
        b = c // 4
        g = c % 4
        lo = GH * g
        hi = GH * (g + 1)
        xt = np.ascontiguousarray(x[b].T).astype(bf)
        wqt = np.ascontiguousarray(w_qkv[lo:hi, :].T).astype(bf)
        wkt = np.ascontiguousarray(w_qkv[H + lo:H + hi, :].T).astype(bf)
        wvt = np.ascontiguousarray(w_qkv[2 * H + lo:2 * H + hi, :].T).astype(bf)
        bqs = np.ascontiguousarray(
            (b_qkv[lo:hi] * SCALE).astype(f32).reshape(G, P).T)
        bk = np.ascontiguousarray(
            b_qkv[H + lo:H + hi].astype(f32).reshape(G, P).T)
        wot = np.ascontiguousarray(w_out[:, lo:hi].T).astype(bf)
        in_maps.append({"xt": xt, "wqt": wqt, "wkt": wkt, "wvt": wvt,
                        "bqs": bqs, "bk": bk, "wot": wot})
    return in_maps


def kernel(x, w_qkv, b_qkv, w_out, b_out):
    import os
    import sys

    x = np.asarray(x, dtype=np.float32)
    w_qkv = np.asarray(w_qkv, dtype=np.float32)
    b_qkv = np.asarray(b_qkv, dtype=np.float32)
    w_out = np.asarray(w_out, dtype=np.float32)
    b_out = np.asarray(b_out, dtype=np.float32)

    from concourse.bass_utils import run_bass_kernel_spmd

    # NTFF tracing under axon needs the antenv.axon_hooks shim (test.py
    # installs it); without it a stray BASS_TRACE=1 in the environment would
    # crash the run — disable tracing in that case.
    if "antenv.axon_hooks" not in sys.modules:
        os.environ["BASS_NEVER_TRACE"] = "1"

    nc = _get_nc()
    in_maps = _make_in_maps(x, w_qkv, b_qkv, w_out)
    res = run_bass_kernel_spmd(nc, in_maps, core_ids=list(range(N_CORES)))
    _CACHE["last_results"] = res
    partials = [r["partial"] for r in res.results]

    bv = b_qkv[2 * H:3 * H]
    bias = b_out + w_out @ bv          # folded v-bias contribution
    out = np.empty((B, S, H), np.float32)
    for b in range(B):
        acc = partials[4 * b].astype(np.float32)
        for g in range(1, 4):
            acc += partials[4 * b + g].astype(np.float32)
        out[b] = acc + bias
    return out


# revision 18
# speedup vs baseline: 1.0287x; 1.0039x over previous
"""Trainium2 Bass kernel for NoTPAttention (dense transformer block:
fused QKV projection -> multi-head attention -> output projection).

Sharding (8 NeuronCores): core c handles batch b = c // 4 and the 4 heads
g = 4*(c % 4) .. 4*(c % 4)+3 (head-parallel tensor parallelism).  Each core
computes its heads' partial out-projection [S, H] in bf16; the host sums the
4 partials per batch in fp32 and adds the (folded) biases.

Numerics: all matmuls run in bf16 with fp32 PSUM accumulation.  Softmax is
computed without max-subtraction (scores are bounded, |s| < ~3.5) with the
normalization deferred to the attention *output*:
    attnT[d, q] = (sum_k v[k, d] * exp(sT[k, q])) / (sum_k exp(sT[k, q]))
The denominator: the 16 key-tiles of exp(sT) are pairwise-tree-summed on the
vector engine (4 strided in-place bf16 adds), then a single ones-matmul
broadcasts the cross-partition sum - 16x less tensor-engine work than a
full ones-matmul accumulation chain.  The v-bias is dropped in-kernel (it
contributes exactly b_v per row after normalization; the host folds
w_out @ b_v into the output bias).

Schedule: the PE instruction stream is emitted in "mini-steps" that weave
the score matmuls of chunk i between the PV matmuls of chunk i-2 and the
out-projection groups, so the in-order PE queue never stalls on the scalar
engine's exp pacing.  The first two chunks' score matmuls are woven into
the tail of the QKV phase (after their k/q slices complete) so the exp
stream gets a 2-chunk head start before PV consumption begins.  Startup:
8 warm-up matmuls on a memset ones tile lift the PE HAM clock gate to
8/8 before the first DMA-gated projection matmul issues.
"""

import numpy as np
import ml_dtypes

B, S, H = 2, 2048, 2048
NH, HD = 16, 128
P = 128
HT = H // P            # 16 hidden-dim tiles
G = 4                  # heads per core
GH = G * HD            # 512: head-group width per core
SCALE = 1.0 / float(np.sqrt(HD))
N_CORES = 8
XC = 512               # phase-1 x streaming chunk (s elements)
QC = 512               # attention query chunk
KT = S // P            # 16 key tiles
NXC = S // XC          # 4
NQC = S // QC          # 4
NCH = G * NQC          # 16 attention chunks

_CACHE = {}


def _build():
    import concourse.mybir as mybir
    import concourse.tile as tile
    from concourse import bacc

    dt = mybir.dt
    Alu = mybir.AluOpType
    Act = mybir.ActivationFunctionType

    nc = bacc.Bacc("TRN2", target_bir_lowering=False, debug=False,
                   enable_asserts=False)

    xt_d = nc.dram_tensor("xt", [H, S], dt.bfloat16, kind="ExternalInput").ap()
    wqt_d = nc.dram_tensor("wqt", [H, GH], dt.bfloat16, kind="ExternalInput").ap()
    wkt_d = nc.dram_tensor("wkt", [H, GH], dt.bfloat16, kind="ExternalInput").ap()
    wvt_d = nc.dram_tensor("wvt", [H, GH], dt.bfloat16, kind="ExternalInput").ap()
    bqs_d = nc.dram_tensor("bqs", [P, G], dt.float32, kind="ExternalInput").ap()
    bk_d = nc.dram_tensor("bk", [P, G], dt.float32, kind="ExternalInput").ap()
    wot_d = nc.dram_tensor("wot", [GH, H], dt.bfloat16, kind="ExternalInput").ap()
    out_d = nc.dram_tensor("partial", [S, H], dt.bfloat16, kind="ExternalOutput").ap()

    xt_r = xt_d.rearrange("(ht p) s -> p ht s", p=P)      # [128, 16, 2048]
    wqt_r = wqt_d.rearrange("(ht p) o -> p ht o", p=P)    # [128, 16, 512]
    wkt_r = wkt_d.rearrange("(ht p) o -> p ht o", p=P)
    wvt_r = wvt_d.rearrange("(ht p) o -> p ht o", p=P)
    wot_r = wot_d.rearrange("(g p) o -> p g o", p=P)      # [128, 4, 2048]

    with tile.TileContext(nc) as tc:
        with (
            tc.tile_pool(name="consts", bufs=1) as consts,
            tc.tile_pool(name="wpool", bufs=1) as wpool,
            tc.tile_pool(name="xpool", bufs=2) as xpool,
            tc.tile_pool(name="big", bufs=1) as big,
            tc.tile_pool(name="epool", bufs=4) as epool,
            tc.tile_pool(name="small", bufs=2) as small,
            tc.tile_pool(name="psum", bufs=2, space="PSUM") as psum,
        ):
            # ---- HAM warm-up: get the PE clock gate to 8/8 before the
            # first real (DMA-gated) matmul arrives ----
            ones_sb = consts.tile([P, 512], dt.bfloat16)
            nc.vector.memset(ones_sb[:], 1.0)
            warm_ps = psum.tile([P, 2, QC], dt.float32, tag="st", name="ps")
            for _ in range(8):
                nc.tensor.matmul(warm_ps[:, 0, :], ones_sb[:, 0:P], ones_sb[:],
                                 start=True, stop=True)

            # ---- startup DMAs, critical-path first: the first q-matmul
            # (head 0) needs wq cols 0:128 and x chunk 0 ----
            wq_sb = epool.tile([P, HT, GH], dt.bfloat16, tag="e", name="wq_sb")
            xt0_sb = xpool.tile([P, HT, XC], dt.bfloat16, tag="xt",
                                name="xt0_sb")
            xt1_sb = xpool.tile([P, HT, XC], dt.bfloat16, tag="xt",
                                name="xt1_sb")
            # fine-grained interleave so the q-matmuls chase each transfer:
            # wq head-slice h feeds q(h); x quads feed the ht accumulation
            nc.sync.dma_start(wq_sb[:, :, 0:HD], wqt_r[:, :, 0:HD])
            bqs_sb = consts.tile([P, G], dt.float32)
            nc.sync.dma_start(bqs_sb[:], bqs_d)
            bk_sb = consts.tile([P, G], dt.float32)
            nc.sync.dma_start(bk_sb[:], bk_d)
            for q4 in range(4):
                nc.sync.dma_start(xt0_sb[:, 4 * q4:4 * (q4 + 1), :],
                                  xt_r[:, 4 * q4:4 * (q4 + 1), 0:XC])
            nc.sync.dma_start(wq_sb[:, :, HD:2 * HD], wqt_r[:, :, HD:2 * HD])
            nc.sync.dma_start(wq_sb[:, :, 2 * HD:3 * HD],
                              wqt_r[:, :, 2 * HD:3 * HD])
            nc.sync.dma_start(wq_sb[:, :, 3 * HD:], wqt_r[:, :, 3 * HD:])
            for q4 in range(4):
                nc.sync.dma_start(xt1_sb[:, 4 * q4:4 * (q4 + 1), :],
                                  xt_r[:, 4 * q4:4 * (q4 + 1), XC:2 * XC])
            wk_sb = epool.tile([P, HT, GH], dt.bfloat16, tag="e", name="wk_sb")
            nc.sync.dma_start(wk_sb[:, :, 0:HD], wkt_r[:, :, 0:HD])
            nc.sync.dma_start(wk_sb[:, :, HD:], wkt_r[:, :, HD:])
            wv_sb = epool.tile([P, HT, GH], dt.bfloat16, tag="e", name="wv_sb")
            nc.sync.dma_start(wv_sb[:], wvt_r)

            qt_sb = big.tile([P, G, S], dt.bfloat16)   # q^T, scale+bias applied
            kt_sb = big.tile([P, G, S], dt.bfloat16)   # k^T, bias applied
            v_sb = big.tile([P, KT, GH], dt.bfloat16)  # v natural [s, o]
            at_sb = big.tile([P, G, S], dt.bfloat16)   # attn output^T

            # ---------------- phase-1 building blocks ----------------
            def q_group(xt_sb, xc, h, filler=0):
                sl = slice(xc * XC, (xc + 1) * XC)
                psq = psum.tile([P, 512], dt.float32, tag="mm", name="psq")
                for ht in range(HT):
                    nc.tensor.matmul(psq,
                                     wq_sb[:, ht, h * HD:(h + 1) * HD],
                                     xt_sb[:, ht, :],
                                     start=(ht == 0), stop=(ht == HT - 1))
                    if filler and ht % 4 == 3 and ht < HT - 1:
                        # keep PE duty high while chasing the x/wq DMAs so
                        # the HAM clock gate never re-throttles mid-startup
                        for _ in range(filler):
                            nc.tensor.matmul(warm_ps[:, 1, :],
                                             ones_sb[:, 0:P], ones_sb[:],
                                             start=True, stop=True)
                nc.vector.tensor_scalar(qt_sb[:, h, sl], psq,
                                        SCALE, bqs_sb[:, h:h + 1],
                                        Alu.mult, Alu.add)

            def k_group(xt_sb, xc, h):
                sl = slice(xc * XC, (xc + 1) * XC)
                psk = psum.tile([P, 512], dt.float32, tag="mm", name="psk")
                for ht in range(HT):
                    nc.tensor.matmul(psk,
                                     wk_sb[:, ht, h * HD:(h + 1) * HD],
                                     xt_sb[:, ht, :],
                                     start=(ht == 0), stop=(ht == HT - 1))
                nc.vector.tensor_scalar_add(kt_sb[:, h, sl], psk,
                                            bk_sb[:, h:h + 1])

            def v_group(xt_sb, xc, sv):
                sm = xc * (XC // P) + sv
                psv = psum.tile([P, 512], dt.float32, tag="mm", name="psv")
                for ht in range(HT):
                    nc.tensor.matmul(psv,
                                     xt_sb[:, ht, sv * P:(sv + 1) * P],
                                     wv_sb[:, ht, :],
                                     start=(ht == 0), stop=(ht == HT - 1))
                nc.vector.tensor_copy(out=v_sb[:, sm, :], in_=psv)

            # ---------------- attention building blocks ----------------
            e_tiles = [None] * NCH
            pv_tiles = [None] * NCH
            zrow_tiles = [None] * NCH
            zch_tiles = [None] * NCH
            proj_q = []
            drain_flip = [0]

            def st_pair(idx, ms):
                # two 128-key score matmuls + one batched exp
                h, qc = idx % G, idx // G
                km = 2 * ms
                ps = psum.tile([P, 2, QC], dt.float32, tag="st", name="ps")
                for j in range(2):
                    nc.tensor.matmul(ps[:, j, :],
                                     kt_sb[:, h, (km + j) * P:(km + j + 1) * P],
                                     qt_sb[:, h, qc * QC:(qc + 1) * QC],
                                     start=True, stop=True)
                nc.scalar.activation(e_tiles[idx][:, km:km + 2, :], ps, Act.Exp)

            def pv_pair(idx, ms):
                h = idx % G
                pv, e = pv_tiles[idx], e_tiles[idx]
                for j in range(2):
                    km = 2 * ms + j
                    nc.tensor.matmul(pv, v_sb[:, km, h * HD:(h + 1) * HD],
                                     e[:, km, :],
                                     start=(km == 0), stop=(km == KT - 1))
                if idx >= NCH - 2:
                    # flush chunks: the PE has slack and the DVE is the
                    # flush bottleneck - accumulate the denominator with a
                    # ones-matmul chain into a free st-tag PSUM bank
                    for j in range(2):
                        km = 2 * ms + j
                        nc.tensor.matmul(zch_tiles[idx], ones_sb[:, 0:P],
                                         e[:, km, :],
                                         start=(km == 0), stop=(km == KT - 1))
                else:
                    # level-1 of the denominator tree: fold this pair's odd
                    # tile into the even one after PV is done reading both
                    nc.vector.tensor_add(e[:, 2 * ms, :], e[:, 2 * ms, :],
                                         e[:, 2 * ms + 1, :])

            def tree_fin(idx):
                # levels 2-4 of the in-place pairwise bf16 tree; the final
                # sum lands in a small zrow buffer so the e-tile's last
                # reader is this tree, not the (later) ones-matmul
                e = e_tiles[idx]
                nc.vector.tensor_add(e[:, 0:KT:4, :], e[:, 0:KT:4, :],
                                     e[:, 2:KT:4, :])
                nc.vector.tensor_add(e[:, 0:KT:8, :], e[:, 0:KT:8, :],
                                     e[:, 4:KT:8, :])
                zr = small.tile([P, QC], dt.bfloat16, tag="zr", name="zr")
                nc.vector.tensor_add(zr[:], e[:, 0, :], e[:, 8, :])
                zrow_tiles[idx] = zr

            def zmm_norm(idx):
                # single ones-matmul: cross-partition sum of the tile-summed
                # exps, broadcast to all 128 partitions; then normalize
                h, qc = idx % G, idx // G
                if zch_tiles[idx] is not None:
                    z = zch_tiles[idx]
                else:
                    z = psum.tile([P, QC], dt.float32, tag="mm", name="z")
                    nc.tensor.matmul(z, ones_sb[:, 0:P], zrow_tiles[idx],
                                     start=True, stop=True)
                zi = small.tile([P, QC], dt.float32, tag="zi", name="zi")
                nc.vector.reciprocal_approx_fast(out=zi[:], in_=z)
                nc.vector.tensor_mul(out=at_sb[:, h, qc * QC:(qc + 1) * QC],
                                     in0=pv_tiles[idx], in1=zi[:])
                if h == G - 1:
                    for sv in range(QC // P):
                        for oc in range(H // 512):
                            proj_q.append((qc * (QC // P) + sv, oc))

            def proj_group(alt=False):
                if not proj_q:
                    return
                sm, oc = proj_q.pop(0)
                drain_flip[0] ^= 1
                pp = psum.tile([P, 512], dt.float32, tag="mm", name="pp")
                for g in range(G):
                    nc.tensor.matmul(pp,
                                     at_sb[:, g, sm * P:(sm + 1) * P],
                                     wo_sb[:, g, oc * 512:(oc + 1) * 512],
                                     start=(g == 0), stop=(g == G - 1))
                ob = small.tile([P, 512], dt.bfloat16, tag="ob", bufs=4,
                                name="ob")
                if alt and drain_flip[0]:
                    nc.scalar.copy(ob[:], pp)
                else:
                    nc.vector.tensor_copy(out=ob[:], in_=pp)
                nc.sync.dma_start(
                    out_d[sm * P:(sm + 1) * P, oc * 512:(oc + 1) * 512],
                    ob[:])

            # ---------------- Phase 1: QKV projections ----------------
            # xc0/xc1: q-projections only (need just wq + x), so the PE has
            # ~27us of work before the first k-group needs the wk DMA
            xts = [None] * NXC
            for xc in range(NXC):
                if xc == 0:
                    xts[xc] = xt0_sb
                elif xc == 1:
                    xts[xc] = xt1_sb
                else:
                    xts[xc] = xpool.tile([P, HT, XC], dt.bfloat16, tag="xt",
                                         name="xt_sb")
                    nc.sync.dma_start(xts[xc][:],
                                      xt_r[:, :, xc * XC:(xc + 1) * XC])
                if xc < 2:
                    for h in range(G):
                        fill = 2 if (xc == 0 or h == 0) else 0
                        q_group(xts[xc], xc, h, filler=fill)
                if xc == 1:
                    for x2 in range(2):
                        for h in range(G):
                            k_group(xts[x2], x2, h)
                    for x2 in range(2):
                        for sv in range(XC // P):
                            v_group(xts[x2], x2, sv)
                if xc == 2:
                    for h in range(G):
                        q_group(xts[xc], xc, h)
                    for h in range(G):
                        k_group(xts[xc], xc, h)
                    for sv in range(XC // P):
                        v_group(xts[xc], xc, sv)

            # last x chunk: k first, then weave the first two chunks' score
            # matmuls into the q/v groups so exp gets a head start
            xt3 = xts[NXC - 1]
            e_tiles[0] = epool.tile([P, KT, QC], dt.bfloat16, tag="e",
                                    name="e_sb")
            k_group(xt3, 3, 0)
            for h in range(1, G):
                k_group(xt3, 3, h)
                st_pair(0, 2 * (h - 1))
                st_pair(0, 2 * (h - 1) + 1)
            q_group(xt3, 3, 0)
            st_pair(0, 6)
            st_pair(0, 7)
            for h in range(1, G):
                q_group(xt3, 3, h)
            # wq now dead -> its pool slot is free for e_tiles[1]
            e_tiles[1] = epool.tile([P, KT, QC], dt.bfloat16, tag="e",
                                    name="e_sb")
            for sv in range(XC // P):
                v_group(xt3, 3, sv)
                st_pair(1, 2 * sv)
                st_pair(1, 2 * sv + 1)

            # out-proj weights: needed only from the first proj (~mid-kernel)
            wo_sb = wpool.tile([P, G, H], dt.bfloat16)
            nc.sync.dma_start(wo_sb[:], wot_r)

            # -------- Phase 2+3: mini-step interleaved chunk pipeline --------
            for j in range(2, 18):
                cur = j if j <= NCH - 1 else None
                pvi = j - 2 if j - 2 <= NCH - 1 else None
                zni = j - 3 if 0 <= j - 3 <= NCH - 1 else None
                if cur is not None:
                    e_tiles[cur] = epool.tile([P, KT, QC], dt.bfloat16,
                                              tag="e", name="e_sb")
                if pvi is not None:
                    pv_tiles[pvi] = psum.tile([P, QC], dt.float32, tag="pv",
                                              name="pv")
                    if pvi >= NCH - 2:
                        zt = psum.tile([P, 2, QC], dt.float32, tag="st",
                                       name="zt")
                        zch_tiles[pvi] = zt[:, 0, :]
                for ms in range(8):
                    if cur is not None:
                        st_pair(cur, ms)
                    if pvi is not None:
                        pv_pair(pvi, ms)
                    if ms == 2 and zni is not None:
                        zmm_norm(zni)
                    if ms % 2 == 1:
                        proj_group(alt=(cur is None))
                if pvi is not None and pvi < NCH - 2:
                    tree_fin(pvi)
            proj_group(alt=True)
            proj_group(alt=True)
            zmm_norm(NCH - 1)
            while proj_q:
                proj_group(alt=True)

    nc.compile()
    return nc


def _get_nc():
    if "nc" not in _CACHE:
        _CACHE["nc"] = _build()
    return _CACHE["nc"]


def _make_in_maps(x, w_qkv, b_qkv, w_out):
    bf = ml_dtypes.bfloat16
    f32 = np.float32
    in_maps = []
    for c in range(N_CORES):
        b = c // 4
        g = c % 4
        lo = GH * g
        hi = GH * (g + 1)
        xt = np.ascontiguousarray(x[b].T).astype(bf)
        wqt = np.ascontiguousarray(w_qkv[lo:hi, :].T).astype(bf)
        wkt = np.ascontiguousarray(w_qkv[H + lo:H + hi, :].T).astype(bf)
        wvt = np.ascontiguousarray(w_qkv[2 * H + lo:2 * H + hi, :].T).astype(bf)
        bqs = np.ascontiguousarray(
            (b_qkv[lo:hi] * SCALE).astype(f32).reshape(G, P).T)
        bk = np.ascontiguousarray(
            b_qkv[H + lo:H + hi].astype(f32).reshape(G, P).T)
        wot = np.ascontiguousarray(w_out[:, lo:hi].T).astype(bf)
        in_maps.append({"xt": xt, "wqt": wqt, "wkt": wkt, "wvt": wvt,
                        "bqs": bqs, "bk": bk, "wot": wot})
    return in_maps


def kernel(x, w_qkv, b_qkv, w_out, b_out):
    import os
    import sys

    x = np.asarray(x, dtype=np.float32)
    w_qkv = np.asarray(w_qkv, dtype=np.float32)
    b_qkv = np.asarray(b_qkv, dtype=np.float32)
    w_out = np.asarray(w_out, dtype=np.float32)
    b_out = np.asarray(b_out, dtype=np.float32)

    from concourse.bass_utils import run_bass_kernel_spmd

    # NTFF tracing under axon needs the antenv.axon_hooks shim (test.py
    # installs it); without it a stray BASS_TRACE=1 in the environment would
    # crash the run — disable tracing in that case.
    if "antenv.axon_hooks" not in sys.modules:
        os.environ["BASS_NEVER_TRACE"] = "1"

    nc = _get_nc()
    in_maps = _make_in_maps(x, w_qkv, b_qkv, w_out)
    res = run_bass_kernel_spmd(nc, in_maps, core_ids=list(range(N_CORES)))
    _CACHE["last_results"] = res
    partials = [r["partial"] for r in res.results]

    bv = b_qkv[2 * H:3 * H]
    bias = b_out + w_out @ bv          # folded v-bias contribution
    out = np.empty((B, S, H), np.float32)
    for b in range(B):
        acc = partials[4 * b].astype(np.float32)
        for g in range(1, 4):
            acc += partials[4 * b + g].astype(np.float32)
        out[b] = acc + bias
    return out
